# revision 5
# baseline (speedup 1.0000x reference)
"""Trainium2 Bass kernel for nn_MeanStdStiffRegularizer (segment reduce).

Strategy (8 NeuronCores, data-parallel over edges):
  - Each core gets 1/8 of the edges laid out as [128, 16384].
  - Per 128-edge column, a PE matmul with a bf16 one-hot of (idx & 127) as
    the stationary operand scatters values into 128 PSUM partitions (bins);
    the moving operand carries 4 hi-group masks (idx >> 7) x 7 bf16 value
    streams -> PSUM [128 bins, 28 cols] accumulates all per-segment sums.
  - Value streams: x, log(|x|+eps), log^2 each split into a bf16 pair for
    ~17-bit effective precision, plus an exact count column.
  - One-hots are generated on DVE with a single is_equal op using stride-0
    broadcast access patterns against constant iota tiles.
  - The [512 segments x 7 sums] partials are summed across cores and the
    final mean/std losses are computed on host in float64.
"""

import sys
import types

import numpy as np

N_EDGES = 16777216
NUM_SEG = 512
STRENGTH = 0.01
STD_WEIGHT = 0.5
EPS = 1e-6

N_CORES = 8
P = 128
F_TOT = N_EDGES // N_CORES // P  # 16384 edges per partition per core
F_MACRO = 512
F_CHUNK = 64
N_HI = 4   # idx >> 7 groups
N_ST = 7   # value streams: xa xb la lb qa qb cnt
N_COL = N_HI * N_ST  # 28 psum columns


def _install_ntff_hook():
    """Register the axon NTFF profiling hook (missing antenv.axon_hooks)."""
    if "antenv.axon_hooks" in sys.modules:
        return
    mod = types.ModuleType("antenv.axon_hooks")
    _h = [None]
    mod.set_axon_ntff_profile_hook = lambda h: _h.__setitem__(0, h)
    mod.get_axon_ntff_profile_hook = lambda: _h[0]
    sys.modules["antenv.axon_hooks"] = mod
    try:
        from trn_agent_boot.trn_boot import _ntff_profile_via_ctypes

        mod.set_axon_ntff_profile_hook(
            _ntff_profile_via_ctypes("/opt/axon/libaxon_pjrt.so")
        )
    except Exception:
        pass


_NO_SPLIT_OPCODES = {
    "CollectiveCompute",
}


def _split_sync_waits(bir_json_bytes):
    """Rewrite BIR so no TPB instruction carries more than one sync wait.

    The walrus codegen in this container supports a single sync-wait slot
    per TPB instruction ("Too many sync wait commands" otherwise).  Extra
    waits are hoisted onto EventSemaphore instructions inserted immediately
    before, on the same engine (same issue-gating semantics).
    """
    import json

    j = json.loads(bir_json_bytes)
    n_split = 0
    uid = [0]
    for f in j["functions"]:
        for b in f["blocks"]:
            out = []
            for ins in b["instructions"]:
                si = ins.get("sync_info")
                ow = (si or {}).get("on_wait") or []
                if len(ow) > 1 and ins.get("opcode") not in _NO_SPLIT_OPCODES:
                    for w in ow[:-1]:
                        uid[0] += 1
                        out.append(
                            {
                                "debug": ins.get("debug", 0),
                                "engine": ins["engine"],
                                "ins": [],
                                "name": f"{ins['name']}-wsplit{uid[0]}",
                                "opcode": "EventSemaphore",
                                "outs": [],
                                "sync_info": {"on_update": [], "on_wait": [w]},
                            }
                        )
                    si["on_wait"] = [ow[-1]]
                    n_split += 1
                out.append(ins)
            b["instructions"] = out
    return json.dumps(j).encode(), n_split


def build_nc(f_tot=F_TOT, f_macro=F_MACRO, f_chunk=F_CHUNK, n_cores=N_CORES):
    """Build the per-core Bass program (SPMD: same program on every core)."""
    import concourse.bass as bass
    import concourse.tile as tile
    from concourse import mybir

    f32 = mybir.dt.float32
    bf16 = mybir.dt.bfloat16
    i16 = mybir.dt.int16
    AOP = mybir.AluOpType
    ACT = mybir.ActivationFunctionType

    assert f_tot % f_macro == 0 and f_macro % f_chunk == 0

    nc = bass.Bass(
        "TRN2", target_bir_lowering=False, debug=False, num_devices=n_cores
    )
    x_d = nc.dram_tensor("x", [P, f_tot], f32, kind="ExternalInput")
    idx_d = nc.dram_tensor("idx", [P, f_tot], i16, kind="ExternalInput")
    iota128_d = nc.dram_tensor("iota128", [P, 128], i16, kind="ExternalInput")
    iota4_d = nc.dram_tensor("iota4", [P, N_HI], i16, kind="ExternalInput")
    out_d = nc.dram_tensor("out", [P, N_COL], f32, kind="ExternalOutput")

    n_macro = f_tot // f_macro
    n_chunk = f_macro // f_chunk
    total_mm = n_macro * n_chunk * f_chunk

    with tile.TileContext(nc) as tc:
        with (
            tc.tile_pool(name="const", bufs=1) as cpool,
            tc.tile_pool(name="io", bufs=3) as io,
            tc.tile_pool(name="mid", bufs=2) as mid,
            tc.tile_pool(name="oh", bufs=3) as ohp,
            tc.tile_pool(name="rh", bufs=3) as rhp,
            tc.tile_pool(name="fin", bufs=1) as fin,
            tc.tile_pool(name="acc", bufs=1, space="PSUM") as psum,
        ):
            i128 = cpool.tile([P, 128], i16)
            nc.sync.dma_start(i128[:], iota128_d[:])
            i4 = cpool.tile([P, N_HI], i16)
            nc.sync.dma_start(i4[:], iota4_d[:])
            eps_t = cpool.tile([P, 1], f32)
            nc.vector.memset(eps_t[:], EPS)

            acc = psum.tile([P, N_COL], f32)

            mm_i = 0
            for t in range(n_macro):
                ts = slice(t * f_macro, (t + 1) * f_macro)
                xt = io.tile([P, f_macro], f32, tag="xt")
                nc.sync.dma_start(xt[:], x_d[:, ts])
                ixt = io.tile([P, f_macro], i16, tag="ixt")
                nc.sync.dma_start(ixt[:], idx_d[:, ts])

                lo7 = mid.tile([P, f_macro], i16, tag="lo7")
                nc.vector.tensor_scalar(
                    lo7[:], ixt[:], 127, None, AOP.bitwise_and
                )
                hi2 = mid.tile([P, f_macro], i16, tag="hi2")
                nc.vector.tensor_scalar(
                    hi2[:], ixt[:], 7, None, AOP.logical_shift_right
                )

                ax = mid.tile([P, f_macro], f32, tag="ax")
                nc.scalar.activation(ax[:], xt[:], ACT.Abs)
                lx = mid.tile([P, f_macro], f32, tag="lx")
                nc.scalar.activation(lx[:], ax[:], ACT.Ln, bias=eps_t[:])
                qx = mid.tile([P, f_macro], f32, tag="qx")
                nc.scalar.activation(qx[:], lx[:], ACT.Square)

                v7 = mid.tile([P, f_macro, N_ST], bf16, tag="v7")
                nc.scalar.activation(v7[:, :, 0], xt[:], ACT.Copy)
                nc.vector.tensor_tensor(
                    v7[:, :, 1], xt[:], v7[:, :, 0], AOP.subtract
                )
                nc.scalar.activation(v7[:, :, 2], lx[:], ACT.Copy)
                nc.vector.tensor_tensor(
                    v7[:, :, 3], lx[:], v7[:, :, 2], AOP.subtract
                )
                nc.scalar.activation(v7[:, :, 4], qx[:], ACT.Copy)
                nc.vector.tensor_tensor(
                    v7[:, :, 5], qx[:], v7[:, :, 4], AOP.subtract
                )
                nc.vector.memset(v7[:, :, 6], 1.0)

                m4 = mid.tile([P, f_macro, N_HI], bf16, tag="m4")
                nc.vector.tensor_tensor(
                    m4[:],
                    hi2[:].unsqueeze(2).broadcast_to([P, f_macro, N_HI]),
                    i4[:].unsqueeze(1).broadcast_to([P, f_macro, N_HI]),
                    AOP.is_equal,
                )

                for c in range(n_chunk):
                    cs = slice(c * f_chunk, (c + 1) * f_chunk)
                    oh = ohp.tile([P, f_chunk, 128], bf16, tag="oh")
                    nc.vector.tensor_tensor(
                        oh[:],
                        lo7[:, cs].unsqueeze(2).broadcast_to([P, f_chunk, 128]),
                        i128[:].unsqueeze(1).broadcast_to([P, f_chunk, 128]),
                        AOP.is_equal,
                    )
                    rh = rhp.tile([P, f_chunk, N_HI, N_ST], bf16, tag="rh")
                    nc.vector.tensor_tensor(
                        rh[:],
                        m4[:, cs, :]
                        .unsqueeze(3)
                        .broadcast_to([P, f_chunk, N_HI, N_ST]),
                        v7[:, cs, :]
                        .unsqueeze(2)
                        .broadcast_to([P, f_chunk, N_HI, N_ST]),
                        AOP.mult,
                    )
                    for fi in range(f_chunk):
                        nc.tensor.matmul(
                            acc[:],
                            oh[:, fi, :],
                            rh[:, fi, :, :],
                            start=(mm_i == 0),
                            stop=(mm_i == total_mm - 1),
                        )
                        mm_i += 1

            outsb = fin.tile([P, N_COL], f32)
            nc.vector.tensor_copy(outsb[:], acc[:])
            nc.sync.dma_start(out_d[:], outsb[:])

    return nc


_PROG_CACHE = {}


def _get_prog(f_tot=F_TOT, f_macro=F_MACRO, f_chunk=F_CHUNK):
    key = (f_tot, f_macro, f_chunk)
    if key not in _PROG_CACHE:
        nc = build_nc(f_tot, f_macro, f_chunk)
        fixed, _n = _split_sync_waits(nc.to_json_bytes())
        nc.to_json_bytes = lambda: fixed
        _PROG_CACHE[key] = nc
    return _PROG_CACHE[key]


def _iota_inputs():
    iota128 = np.tile(np.arange(128, dtype=np.int16), (P, 1))
    iota4 = np.tile(np.arange(N_HI, dtype=np.int16), (P, 1))
    return iota128, iota4


def _finale(partials, target_mean, target_std):
    """partials: [512, 7] float64 summed across cores -> scalar loss."""
    xs = partials[:, 0] + partials[:, 1]
    ls = partials[:, 2] + partials[:, 3]
    qs = partials[:, 4] + partials[:, 5]
    cnt = partials[:, 6]
    cg = np.maximum(cnt, 1.0)
    mean_w = xs / cg
    mean_log = ls / cg
    log_var = qs / cg - mean_log**2
    std_w = np.sqrt(log_var + EPS)
    mean_loss = np.mean((mean_w - target_mean.astype(np.float64)) ** 2)
    std_loss = np.mean((std_w - target_std.astype(np.float64)) ** 2)
    total = (1.0 - STD_WEIGHT) * mean_loss + STD_WEIGHT * std_loss
    return np.float32(total * STRENGTH)


def run_partials(x, idx, trace=False):
    """Run the device program; return per-core [128, 28] partials summed."""
    _install_ntff_hook()
    from concourse.bass_utils import run_bass_kernel_spmd

    nc = _get_prog()
    iota128, iota4 = _iota_inputs()
    per_core = N_EDGES // N_CORES
    in_maps = []
    for c in range(N_CORES):
        sl = slice(c * per_core, (c + 1) * per_core)
        in_maps.append(
            {
                "x": np.ascontiguousarray(
                    x[sl].reshape(P, F_TOT), dtype=np.float32
                ),
                "idx": idx[sl].reshape(P, F_TOT).astype(np.int16),
                "iota128": iota128,
                "iota4": iota4,
            }
        )
    res = run_bass_kernel_spmd(
        nc, in_maps, list(range(N_CORES)), trace=trace
    )
    # out[b, g*7+j] holds sums for segment s = g*128 + b, stream j.
    partials = np.zeros((NUM_SEG, N_ST), dtype=np.float64)
    for c in range(N_CORES):
        o = res.results[c]["out"].astype(np.float64)  # [128, 28]
        partials += o.reshape(P, N_HI, N_ST).transpose(1, 0, 2).reshape(
            NUM_SEG, N_ST
        )
    return partials, res


def kernel(x, idx, target_mean, target_std):
    partials, _res = run_partials(x, idx, trace=False)
    return _finale(partials, target_mean, target_std)


# revision 9
# speedup vs baseline: 1.1948x; 1.1948x over previous
"""Trainium2 Bass kernel for nn_MeanStdStiffRegularizer (segment reduce).

Strategy (8 NeuronCores, data-parallel over edges):
  - Each core gets 1/8 of the edges laid out as [128, 16384].
  - Per 128-edge column f, a PE matmul scatters values into PSUM bins:
      lhsT = one-hot of (idx & 63)   -> 64 PSUM partitions (bins)
      rhs  = 8 hi-group masks (idx >> 6) x 4 value streams -> 32 psum cols
    PSUM [64, 32] accumulates every per-segment sum for 512 segments.
  - The bin one-hot is built TRANSPOSED ([P, 64, F]) with 64 tensor_scalar
    is_equal ops (contiguous step-1 16-bit in/out -> DVE 4x mode); the rhs
    is built as 32 contiguous tensor_tensor mults (2x mode).  The matmul
    reads both operands with strided column APs.
  - Value streams: bf16 x, log(|x|+eps), log^2, count (exact).
  - The [512 segments x 4 sums] partials are summed across cores and the
    final mean/std losses are computed on host in float64.
"""

import sys
import types

import numpy as np

N_EDGES = 16777216
NUM_SEG = 512
STRENGTH = 0.01
STD_WEIGHT = 0.5
EPS = 1e-6

N_CORES = 8
P = 128
F_TOT = N_EDGES // N_CORES // P  # 16384 edges per partition per core
F_MACRO = 512
N_BIN = 64   # idx & 63 -> psum partitions
N_HI = 8     # idx >> 6 -> rhs groups
N_ST = 4     # value streams: x, log, log^2, count
N_COL = N_HI * N_ST  # 32 psum columns


def _install_ntff_hook():
    """Register the axon NTFF profiling hook (missing antenv.axon_hooks)."""
    if "antenv.axon_hooks" in sys.modules:
        return
    mod = types.ModuleType("antenv.axon_hooks")
    _h = [None]
    mod.set_axon_ntff_profile_hook = lambda h: _h.__setitem__(0, h)
    mod.get_axon_ntff_profile_hook = lambda: _h[0]
    sys.modules["antenv.axon_hooks"] = mod
    try:
        from trn_agent_boot.trn_boot import _ntff_profile_via_ctypes

        mod.set_axon_ntff_profile_hook(
            _ntff_profile_via_ctypes("/opt/axon/libaxon_pjrt.so")
        )
    except Exception:
        pass


_NO_SPLIT_OPCODES = {
    "CollectiveCompute",
}


def _split_sync_waits(bir_json_bytes):
    """Rewrite BIR so no TPB instruction carries more than one sync wait.

    The walrus codegen in this container supports a single sync-wait slot
    per TPB instruction ("Too many sync wait commands" otherwise).  Extra
    waits are hoisted onto EventSemaphore instructions inserted immediately
    before, on the same engine (same issue-gating semantics).
    """
    import json

    j = json.loads(bir_json_bytes)
    n_split = 0
    uid = [0]
    for f in j["functions"]:
        for b in f["blocks"]:
            out = []
            for ins in b["instructions"]:
                si = ins.get("sync_info")
                ow = (si or {}).get("on_wait") or []
                if len(ow) > 1 and ins.get("opcode") not in _NO_SPLIT_OPCODES:
                    for w in ow[:-1]:
                        uid[0] += 1
                        out.append(
                            {
                                "debug": ins.get("debug", 0),
                                "engine": ins["engine"],
                                "ins": [],
                                "name": f"{ins['name']}-wsplit{uid[0]}",
                                "opcode": "EventSemaphore",
                                "outs": [],
                                "sync_info": {"on_update": [], "on_wait": [w]},
                            }
                        )
                    si["on_wait"] = [ow[-1]]
                    n_split += 1
                out.append(ins)
            b["instructions"] = out
    return json.dumps(j).encode(), n_split


def build_nc(f_tot=F_TOT, f_macro=F_MACRO, n_cores=N_CORES):
    """Build the per-core Bass program (SPMD: same program on every core)."""
    import concourse.bass as bass
    import concourse.tile as tile
    from concourse import mybir

    f32 = mybir.dt.float32
    bf16 = mybir.dt.bfloat16
    i16 = mybir.dt.int16
    AOP = mybir.AluOpType
    ACT = mybir.ActivationFunctionType

    assert f_tot % f_macro == 0

    nc = bass.Bass(
        "TRN2", target_bir_lowering=False, debug=False, num_devices=n_cores
    )
    x_d = nc.dram_tensor("x", [P, f_tot], f32, kind="ExternalInput")
    idx_d = nc.dram_tensor("idx", [P, f_tot], i16, kind="ExternalInput")
    out_d = nc.dram_tensor("out", [N_BIN, N_COL], f32, kind="ExternalOutput")

    n_macro = f_tot // f_macro
    total_mm = n_macro * f_macro

    with tile.TileContext(nc) as tc:
        with (
            tc.tile_pool(name="const", bufs=1) as cpool,
            tc.tile_pool(name="io", bufs=2) as io,
            tc.tile_pool(name="mid", bufs=2) as mid,
            tc.tile_pool(name="oh", bufs=2) as ohp,
            tc.tile_pool(name="rh", bufs=2) as rhp,
            tc.tile_pool(name="fin", bufs=1) as fin,
            tc.tile_pool(name="acc", bufs=1, space="PSUM") as psum,
        ):
            eps_t = cpool.tile([P, 1], f32)
            nc.vector.memset(eps_t[:], EPS)

            acc = psum.tile([N_BIN, N_COL], f32)

            mm_i = 0
            for t in range(n_macro):
                ts = slice(t * f_macro, (t + 1) * f_macro)
                xt = io.tile([P, f_macro], f32, tag="xt")
                nc.sync.dma_start(xt[:], x_d[:, ts])
                ixt = io.tile([P, f_macro], i16, tag="ixt")
                nc.sync.dma_start(ixt[:], idx_d[:, ts])

                lo6 = mid.tile([P, f_macro], i16, tag="lo6")
                nc.vector.tensor_scalar(
                    lo6[:], ixt[:], 63, None, AOP.bitwise_and
                )
                hi3 = mid.tile([P, f_macro], i16, tag="hi3")
                nc.vector.tensor_scalar(
                    hi3[:], ixt[:], 6, None, AOP.logical_shift_right
                )

                ax = mid.tile([P, f_macro], f32, tag="ax")
                nc.scalar.activation(ax[:], xt[:], ACT.Abs)
                lx = mid.tile([P, f_macro], f32, tag="lx")
                nc.scalar.activation(lx[:], ax[:], ACT.Ln, bias=eps_t[:])

                # value streams, stream-major: vv[:, j, :] contiguous
                vv = mid.tile([P, N_ST, f_macro], bf16, tag="vv")
                nc.scalar.activation(vv[:, 0, :], xt[:], ACT.Copy)
                nc.scalar.activation(vv[:, 1, :], lx[:], ACT.Copy)
                nc.vector.tensor_tensor(
                    vv[:, 2, :], vv[:, 1, :], vv[:, 1, :], AOP.mult
                )
                nc.vector.memset(vv[:, 3, :], 1.0)

                # hi-group masks, group-major: m8[:, g, :] contiguous
                m8 = mid.tile([P, N_HI, f_macro], bf16, tag="m8")
                for g in range(N_HI):
                    nc.vector.tensor_scalar(
                        m8[:, g, :], hi3[:], g, None, AOP.is_equal
                    )

                # transposed one-hot of lo6: ohT[:, b, :] contiguous
                # (64 tensor_scalar is_equal ops -> DVE 4x mode)
                ohT = ohp.tile([P, N_BIN, f_macro], bf16, tag="ohT")
                for b in range(N_BIN):
                    nc.vector.tensor_scalar(
                        ohT[:, b, :], lo6[:], b, None, AOP.is_equal
                    )

                # rhs, (g, j)-major: rh[:, g, j, :] contiguous
                # (32 contiguous tensor_tensor mults -> DVE 2x mode),
                # chunked along f to bound SBUF usage
                f_chunk = f_macro // 2 if f_macro >= 512 else f_macro
                for c0 in range(0, f_macro, f_chunk):
                    cs = slice(c0, c0 + f_chunk)
                    rh = rhp.tile([P, N_HI, N_ST, f_chunk], bf16, tag="rh")
                    for g in range(N_HI):
                        for jj in range(N_ST):
                            nc.vector.tensor_tensor(
                                rh[:, g, jj, :],
                                m8[:, g, cs],
                                vv[:, jj, cs],
                                AOP.mult,
                            )

                    for fi in range(f_chunk):
                        nc.tensor.matmul(
                            acc[:],
                            ohT[:, :, c0 + fi],
                            rh[:, :, :, fi],
                            start=(mm_i == 0),
                            stop=(mm_i == total_mm - 1),
                        )
                        mm_i += 1

            outsb = fin.tile([N_BIN, N_COL], f32)
            nc.vector.tensor_copy(outsb[:], acc[:])
            nc.sync.dma_start(out_d[:], outsb[:])

    return nc


_PROG_CACHE = {}


def _get_prog(f_tot=F_TOT, f_macro=F_MACRO):
    key = (f_tot, f_macro)
    if key not in _PROG_CACHE:
        nc = build_nc(f_tot, f_macro)
        fixed, _n = _split_sync_waits(nc.to_json_bytes())
        nc.to_json_bytes = lambda: fixed
        _PROG_CACHE[key] = nc
    return _PROG_CACHE[key]


def _finale(partials, target_mean, target_std):
    """partials: [512, 4] float64 summed across cores -> scalar loss."""
    xs = partials[:, 0]
    ls = partials[:, 1]
    qs = partials[:, 2]
    cnt = partials[:, 3]
    cg = np.maximum(cnt, 1.0)
    mean_w = xs / cg
    mean_log = ls / cg
    log_var = qs / cg - mean_log**2
    std_w = np.sqrt(log_var + EPS)
    mean_loss = np.mean((mean_w - target_mean.astype(np.float64)) ** 2)
    std_loss = np.mean((std_w - target_std.astype(np.float64)) ** 2)
    total = (1.0 - STD_WEIGHT) * mean_loss + STD_WEIGHT * std_loss
    return np.float32(total * STRENGTH)


def run_partials(x, idx, trace=False):
    """Run the device program; return [512, 4] partials summed over cores."""
    _install_ntff_hook()
    from concourse.bass_utils import run_bass_kernel_spmd

    nc = _get_prog()
    per_core = N_EDGES // N_CORES
    in_maps = []
    for c in range(N_CORES):
        sl = slice(c * per_core, (c + 1) * per_core)
        in_maps.append(
            {
                "x": np.ascontiguousarray(
                    x[sl].reshape(P, F_TOT), dtype=np.float32
                ),
                "idx": idx[sl].reshape(P, F_TOT).astype(np.int16),
            }
        )
    res = run_bass_kernel_spmd(
        nc, in_maps, list(range(N_CORES)), trace=trace
    )
    # out[b, g*N_ST+j] holds sums for segment s = g*64 + b, stream j.
    partials = np.zeros((NUM_SEG, N_ST), dtype=np.float64)
    for c in range(N_CORES):
        o = res.results[c]["out"].astype(np.float64)  # [64, 32]
        partials += o.reshape(N_BIN, N_HI, N_ST).transpose(1, 0, 2).reshape(
            NUM_SEG, N_ST
        )
    return partials, res


def kernel(x, idx, target_mean, target_std):
    partials, _res = run_partials(x, idx, trace=False)
    return _finale(partials, target_mean, target_std)


# revision 14
# speedup vs baseline: 1.6392x; 1.3719x over previous
"""Trainium2 Bass kernel for nn_MeanStdStiffRegularizer (segment reduce).

Strategy (8 NeuronCores, data-parallel over edges):
  - Each core gets 1/8 of the edges laid out as [128, 16384].
  - Per 128-edge column f, a PE matmul scatters values into PSUM bins:
      lhsT = one-hot of (idx & 63)   -> 64 PSUM partitions (bins)
      rhs  = 8 hi-group masks (idx >> 6) x 4 value streams -> 32 psum cols
    PSUM [64, 32] accumulates every per-segment sum for 512 segments.
  - The bin one-hot is built TRANSPOSED ([P, 64, F]) with 64 tensor_scalar
    is_equal ops (contiguous step-1 16-bit in/out -> DVE 4x mode); the rhs
    is built as 32 contiguous tensor_tensor mults (2x mode).  The matmul
    reads both operands with strided column APs.
  - Value streams: bf16 x, log(|x|+eps), log^2, count (exact).
  - The [512 segments x 4 sums] partials are summed across cores and the
    final mean/std losses are computed on host in float64.
"""

import sys
import types

import numpy as np

N_EDGES = 16777216
NUM_SEG = 512
STRENGTH = 0.01
STD_WEIGHT = 0.5
EPS = 1e-6

N_CORES = 8
P = 128
F_TOT = N_EDGES // N_CORES // P  # 16384 edges per partition per core
F_MACRO = 512
N_BIN = 64   # idx & 63 -> psum partitions
N_HI = 8     # idx >> 6 -> rhs groups
N_ST = 4     # value streams: x, log, log^2, count
N_COL = N_HI * N_ST  # 32 psum columns


def _install_ntff_hook():
    """Register the axon NTFF profiling hook (missing antenv.axon_hooks)."""
    if "antenv.axon_hooks" in sys.modules:
        return
    mod = types.ModuleType("antenv.axon_hooks")
    _h = [None]
    mod.set_axon_ntff_profile_hook = lambda h: _h.__setitem__(0, h)
    mod.get_axon_ntff_profile_hook = lambda: _h[0]
    sys.modules["antenv.axon_hooks"] = mod
    try:
        from trn_agent_boot.trn_boot import _ntff_profile_via_ctypes

        mod.set_axon_ntff_profile_hook(
            _ntff_profile_via_ctypes("/opt/axon/libaxon_pjrt.so")
        )
    except Exception:
        pass


_NO_SPLIT_OPCODES = {
    "CollectiveCompute",
}


def _split_sync_waits(bir_json_bytes):
    """Rewrite BIR so no TPB instruction carries more than one sync wait.

    The walrus codegen in this container supports a single sync-wait slot
    per TPB instruction ("Too many sync wait commands" otherwise).  Extra
    waits are hoisted onto EventSemaphore instructions inserted immediately
    before, on the same engine (same issue-gating semantics).
    """
    import json

    j = json.loads(bir_json_bytes)
    n_split = 0
    uid = [0]
    for f in j["functions"]:
        for b in f["blocks"]:
            out = []
            for ins in b["instructions"]:
                si = ins.get("sync_info")
                ow = (si or {}).get("on_wait") or []
                if len(ow) > 1 and ins.get("opcode") not in _NO_SPLIT_OPCODES:
                    for w in ow[:-1]:
                        uid[0] += 1
                        out.append(
                            {
                                "debug": ins.get("debug", 0),
                                "engine": ins["engine"],
                                "ins": [],
                                "name": f"{ins['name']}-wsplit{uid[0]}",
                                "opcode": "EventSemaphore",
                                "outs": [],
                                "sync_info": {"on_update": [], "on_wait": [w]},
                            }
                        )
                    si["on_wait"] = [ow[-1]]
                    n_split += 1
                out.append(ins)
            b["instructions"] = out
    return json.dumps(j).encode(), n_split


def build_nc(f_tot=F_TOT, f_macro=F_MACRO, n_cores=N_CORES):
    """Build the per-core Bass program (SPMD: same program on every core)."""
    import concourse.bass as bass
    import concourse.tile as tile
    from concourse import mybir

    f32 = mybir.dt.float32
    bf16 = mybir.dt.bfloat16
    i16 = mybir.dt.int16
    AOP = mybir.AluOpType
    ACT = mybir.ActivationFunctionType

    assert f_tot % f_macro == 0

    nc = bass.Bass(
        "TRN2", target_bir_lowering=False, debug=False, num_devices=n_cores
    )
    x_d = nc.dram_tensor("x", [P, f_tot], f32, kind="ExternalInput")
    idx_d = nc.dram_tensor("idx", [P, f_tot], i16, kind="ExternalInput")
    out_d = nc.dram_tensor("out", [N_COL, N_BIN], f32, kind="ExternalOutput")

    n_macro = f_tot // f_macro
    total_mm = n_macro * f_macro

    with tile.TileContext(nc) as tc:
        with (
            tc.tile_pool(name="const", bufs=1) as cpool,
            tc.tile_pool(name="io", bufs=2) as io,
            tc.tile_pool(name="mid", bufs=2) as mid,
            tc.tile_pool(name="oh", bufs=2) as ohp,
            tc.tile_pool(name="rh", bufs=2) as rhp,
            tc.tile_pool(name="fin", bufs=1) as fin,
            tc.tile_pool(name="acc", bufs=1, space="PSUM") as psum,
        ):
            eps_t = cpool.tile([P, 1], f32)
            nc.vector.memset(eps_t[:], EPS)

            acc = psum.tile([N_COL, N_BIN], f32)

            mm_i = 0
            for t in range(n_macro):
                ts = slice(t * f_macro, (t + 1) * f_macro)
                xt = io.tile([P, f_macro], f32, tag="xt")
                nc.sync.dma_start(xt[:], x_d[:, ts])
                ixt = io.tile([P, f_macro], i16, tag="ixt")
                nc.sync.dma_start(ixt[:], idx_d[:, ts])

                lo6 = mid.tile([P, f_macro], i16, tag="lo6")
                nc.vector.tensor_scalar(
                    lo6[:], ixt[:], 63, None, AOP.bitwise_and
                )
                hi3 = mid.tile([P, f_macro], i16, tag="hi3")
                nc.vector.tensor_scalar(
                    hi3[:], ixt[:], 6, None, AOP.logical_shift_right
                )

                ax = mid.tile([P, f_macro], f32, tag="ax")
                nc.scalar.activation(ax[:], xt[:], ACT.Abs)
                lx = mid.tile([P, f_macro], f32, tag="lx")
                nc.scalar.activation(lx[:], ax[:], ACT.Ln, bias=eps_t[:])

                # value streams, stream-major: vv[:, j, :] contiguous
                vv = mid.tile([P, N_ST, f_macro], bf16, tag="vv")
                nc.scalar.activation(vv[:, 0, :], xt[:], ACT.Copy)
                nc.scalar.activation(vv[:, 1, :], lx[:], ACT.Copy)
                nc.vector.tensor_tensor(
                    vv[:, 2, :], vv[:, 1, :], vv[:, 1, :], AOP.mult
                )
                nc.vector.memset(vv[:, 3, :], 1.0)

                # hi-group masks, group-major: m8[:, g, :] contiguous
                m8 = mid.tile([P, N_HI, f_macro], bf16, tag="m8")
                for g in range(N_HI):
                    nc.vector.tensor_scalar(
                        m8[:, g, :], hi3[:], g, None, AOP.is_equal
                    )

                # transposed one-hot of lo6: ohT[:, b, :] contiguous
                # (64 tensor_scalar is_equal ops -> DVE 4x mode)
                ohT = ohp.tile([P, N_BIN, f_macro], bf16, tag="ohT")
                for b in range(N_BIN):
                    nc.vector.tensor_scalar(
                        ohT[:, b, :], lo6[:], b, None, AOP.is_equal
                    )

                # rhs values, f-major: rh[:, f, g, j] so the matmul's
                # stationary operand rh[:, fi, :, :] is contiguous.
                # Built with kron-broadcast APs (1x DVE); part of the work
                # is offloaded to the otherwise-idle GpSimd engine.
                f_chunk = f_macro // 2 if f_macro >= 512 else f_macro
                for c0 in range(0, f_macro, f_chunk):
                    cs = slice(c0, c0 + f_chunk)
                    rh = rhp.tile([P, f_chunk, N_HI, N_ST], bf16, tag="rh")
                    half = f_chunk // 2
                    for eng, hs, os_ in (
                        (nc.vector, slice(c0, c0 + half), slice(0, half)),
                        (
                            nc.gpsimd,
                            slice(c0 + half, c0 + f_chunk),
                            slice(half, f_chunk),
                        ),
                    ):
                        n_f = hs.stop - hs.start
                        eng.tensor_tensor(
                            rh[:, os_, :, :],
                            m8[:, :, hs]
                            .rearrange("p g f -> p f g")
                            .unsqueeze(3)
                            .broadcast_to([P, n_f, N_HI, N_ST]),
                            vv[:, :, hs]
                            .rearrange("p j f -> p f j")
                            .unsqueeze(2)
                            .broadcast_to([P, n_f, N_HI, N_ST]),
                            AOP.mult,
                        )

                    for fi in range(f_chunk):
                        nc.tensor.matmul(
                            acc[:],
                            rh[:, fi, :, :],
                            ohT[:, :, c0 + fi],
                            start=(mm_i == 0),
                            stop=(mm_i == total_mm - 1),
                        )
                        mm_i += 1

            outsb = fin.tile([N_COL, N_BIN], f32)
            nc.vector.tensor_copy(outsb[:], acc[:])
            nc.sync.dma_start(out_d[:], outsb[:])

    return nc


_PROG_CACHE = {}


def _get_prog(f_tot=F_TOT, f_macro=F_MACRO):
    key = (f_tot, f_macro)
    if key not in _PROG_CACHE:
        nc = build_nc(f_tot, f_macro)
        fixed, _n = _split_sync_waits(nc.to_json_bytes())
        nc.to_json_bytes = lambda: fixed
        _PROG_CACHE[key] = nc
    return _PROG_CACHE[key]


def _finale(partials, target_mean, target_std):
    """partials: [512, 4] float64 summed across cores -> scalar loss."""
    xs = partials[:, 0]
    ls = partials[:, 1]
    qs = partials[:, 2]
    cnt = partials[:, 3]
    cg = np.maximum(cnt, 1.0)
    mean_w = xs / cg
    mean_log = ls / cg
    log_var = qs / cg - mean_log**2
    std_w = np.sqrt(log_var + EPS)
    mean_loss = np.mean((mean_w - target_mean.astype(np.float64)) ** 2)
    std_loss = np.mean((std_w - target_std.astype(np.float64)) ** 2)
    total = (1.0 - STD_WEIGHT) * mean_loss + STD_WEIGHT * std_loss
    return np.float32(total * STRENGTH)


def run_partials(x, idx, trace=False):
    """Run the device program; return [512, 4] partials summed over cores."""
    _install_ntff_hook()
    from concourse.bass_utils import run_bass_kernel_spmd

    nc = _get_prog()
    per_core = N_EDGES // N_CORES
    in_maps = []
    for c in range(N_CORES):
        sl = slice(c * per_core, (c + 1) * per_core)
        in_maps.append(
            {
                "x": np.ascontiguousarray(
                    x[sl].reshape(P, F_TOT), dtype=np.float32
                ),
                "idx": idx[sl].reshape(P, F_TOT).astype(np.int16),
            }
        )
    res = run_bass_kernel_spmd(
        nc, in_maps, list(range(N_CORES)), trace=trace
    )
    # out[g*N_ST+j, b] holds sums for segment s = g*64 + b, stream j.
    partials = np.zeros((NUM_SEG, N_ST), dtype=np.float64)
    for c in range(N_CORES):
        o = res.results[c]["out"].astype(np.float64)  # [32, 64]
        partials += o.reshape(N_HI, N_ST, N_BIN).transpose(0, 2, 1).reshape(
            NUM_SEG, N_ST
        )
    return partials, res


def kernel(x, idx, target_mean, target_std):
    partials, _res = run_partials(x, idx, trace=False)
    return _finale(partials, target_mean, target_std)
